# revision 1
# baseline (speedup 1.0000x reference)
"""DynamicEdgeConv GNN (3x EdgeConv + encoder) on 8 TRN2 NeuronCores.

Data-parallel over graphs: 16 graphs of 2048 nodes; 2 graphs per core.
Per graph-conv: hT [H=128, N=2048] kept feature-major in SBUF.
  scores(i,j) = h_i . h_j - 0.5*||h_j||^2   (argtop8 == kNN by distance)
  top-8 via DVE max / max_index, gather rows via indirect DMA from a DRAM
  copy of h, message MLP via PE with the [xi, xj-xi] concat rewritten as
  xi@(A-B) + xj@B, max-aggregate over k via DVE tensor_reduce on a strided
  view. Bias of the encoder is folded as a K=5 matmul; -0.5||h_j||^2 is
  folded as a K=1 ones matmul into the scores PSUM group.
"""

import numpy as np
from contextlib import ExitStack

import concourse.bass as bass
import concourse.mybir as mybir
from concourse import tile
from concourse.masks import make_identity

B_ALL = 16      # graphs total
N = 2048        # nodes per graph
KNN = 8
H = 128
F_IN = 4
CORES = 8
GPC = B_ALL // CORES          # graphs per core
NPC = GPC * N                 # nodes per core
NCH = N // 128                # 16 chunks of 128 nodes per graph
NB = N // 512                 # 4 blocks of 512 nodes per graph

FP = mybir.dt.float32
FR = mybir.dt.float32r
F16 = mybir.dt.float16
U32 = mybir.dt.uint32


def fp(ap):
    return ap.bitcast(FP)


# weights consumed as f32r matmul operands (DMA'd straight into f32r tiles);
# biases consumed by ACT stay fp32
FR_WEIGHTS = {"W_enc", "b_enc", "W1a", "W1b", "W2a", "W2b", "W5a", "W5b"}
AF = mybir.ActivationFunctionType
ALU = mybir.AluOpType
AX = mybir.AxisListType

CONV_TAGS = ["1", "2", "5"]

WEIGHT_SPECS = {
    "W_enc": (F_IN, H), "b_enc": (1, H),
    "W1a": (2 * H, H), "b1a": (H, 1), "W1b": (H, H), "b1b": (H, 1),
    "W2a": (2 * H, H), "b2a": (H, 1), "W2b": (H, H), "b2b": (H, 1),
    "W5a": (2 * H, H), "b5a": (H, 1), "W5b": (H, 1), "b5b": (1, 1),
}


def emit(tc, x, out_d, W):
    nc = tc.nc
    with ExitStack() as ctx:
        consts = ctx.enter_context(tc.tile_pool(name="consts", bufs=1))
        hpool = ctx.enter_context(tc.tile_pool(name="hpool", bufs=3))
        work = ctx.enter_context(tc.tile_pool(name="work", bufs=2))
        # deep pool for the small per-k MLP tiles: the k-chain is
        # latency-bound, so 4-deep rotation lets k+2/k+3 start early
        mlpp = ctx.enter_context(tc.tile_pool(name="mlpp", bufs=4))
        strips = ctx.enter_context(tc.tile_pool(name="strips", bufs=1))
        psum = ctx.enter_context(tc.tile_pool(name="psum", bufs=2, space="PSUM"))
        hdram = ctx.enter_context(tc.tile_pool(name="hdram", bufs=1, space="DRAM"))

        ident = consts.tile([128, 128], FP, tag="ident", name="ident")
        make_identity(nc, ident)
        ones_cf = consts.tile([128, 1], FP, tag="ones_cf", name="ones_cf")
        nc.vector.memset(ones_cf, 1.0)
        ones_col = consts.tile([128, 1], FR, tag="ones_col", name="ones_col")
        nc.scalar.activation(ones_col, ones_cf, AF.Copy)
        ones_5f = consts.tile([1, 512], FP, tag="ones_5f", name="ones_5f")
        nc.vector.memset(ones_5f, 1.0)
        ones_512 = consts.tile([1, 512], FR, tag="ones_512", name="ones_512")
        nc.scalar.activation(ones_512, ones_5f, AF.Copy)
        ones_row = consts.tile([1, 128], FR, tag="ones_row", name="ones_row")
        nc.scalar.activation(ones_row, ones_5f[:, 0:128], AF.Copy)

        w_enc_sb = consts.tile([F_IN, H], FR, tag="w_enc_sb", name="w_enc_sb")
        nc.sync.dma_start(w_enc_sb, W["W_enc"])
        b_enc_sb = consts.tile([1, H], FR, tag="b_enc_sb", name="b_enc_sb")
        nc.sync.dma_start(b_enc_sb, W["b_enc"])

        convW = []
        for t in CONV_TAGS:
            AB = consts.tile([H, 2 * H], FR, tag=f"AB{t}", name=f"AB{t}")
            nc.sync.dma_start(AB.rearrange("h (a j) -> h a j", a=2),
                              W[f"W{t}a"].rearrange("(a h) j -> h a j", a=2))
            Bm = AB[:, H:2 * H]
            AmB = consts.tile([H, H], FR, tag=f"AmB{t}", name=f"AmB{t}")
            nc.vector.tensor_sub(AmB, AB[:, 0:H], Bm)
            ba = consts.tile([H, 1], FP, tag=f"ba{t}", name=f"ba{t}")
            nc.sync.dma_start(ba, W[f"b{t}a"])
            if t != "5":
                Wb = consts.tile([H, H], FR, tag=f"Wb{t}", name=f"Wb{t}")
                bb = consts.tile([H, 1], FP, tag=f"bb{t}", name=f"bb{t}")
            else:
                Wb = consts.tile([H, 1], FR, tag=f"Wb{t}", name=f"Wb{t}")
                bb = consts.tile([1, 1], FP, tag=f"bb{t}", name=f"bb{t}")
            nc.sync.dma_start(Wb, W[f"W{t}b"])
            nc.sync.dma_start(bb, W[f"b{t}b"])
            convW.append((AmB, Bm, ba, Wb, bb))

        h_tab = [[hdram.tile([N, H], FP, tag=f"ht_{g}_{c}", name=f"ht_{g}_{c}")
                  for c in range(3)] for g in range(GPC)]

        # x transposed into SBUF (feature-major)
        xT = consts.tile([F_IN, NPC], FR, tag="xT", name="xT")
        nc.sync.dma_start(xT, x.rearrange("n f -> f n"))

        def store_htab(g, c, hT_src):
            dst = h_tab[g][c].rearrange("(cb q p) f -> cb p q f", q=4, p=128)
            for cb in range(4):
                pst = psum.tile([128, 512], FP, tag="t", name="pst_st")
                for q in range(4):
                    col = (cb * 4 + q) * 128
                    nc.tensor.transpose(pst[:, q * 128:(q + 1) * 128],
                                        fp(hT_src[:, col:col + 128]), ident)
                hsb = work.tile([128, 512], FP, tag="hst", name="hsb")
                nc.scalar.activation(hsb, pst, AF.Copy)
                nc.sync.dma_start(dst[cb], hsb.rearrange("p (q f) -> p q f", q=4))

        def edge_conv(g, conv, hT_in):
            AmB, Bm, ba, Wb, bb = convW[conv]

            h2 = work.tile([H, N], FR, tag="h2", name="h2")
            nc.scalar.activation(h2, fp(hT_in), AF.Square)
            neghalf = strips.tile([1, N], FR, tag="nh", name="neghalf")
            for jb in range(NB):
                ps = psum.tile([128, 512], FP, tag="s", name="ps_sq")
                nc.tensor.matmul(ps[0:1, :], ones_col,
                                 h2[:, jb * 512:(jb + 1) * 512],
                                 start=True, stop=True)
                nc.scalar.activation(neghalf[:, jb * 512:(jb + 1) * 512], ps[0:1, :],
                                     AF.Copy, scale=-0.5)

            # unique idx tile per (g, conv): avoids WAR waits from the 8
            # SWDGE gather queues landing on max_index (1-wait-slot limit)
            idx = consts.tile([128, NCH * KNN], U32, tag=f"idx_{g}_{conv}",
                              name=f"idx_{g}_{conv}")
            def emit_scores(ib):
                for q in range(4):
                    ci = ib * 4 + q
                    # fp16 scores: halves the DVE top-8 scan cost; only the
                    # argmax selection consumes these values
                    sc = work.tile([128, N], F16, tag="sc", name="sc")
                    for jb in range(NB):
                        ps = psum.tile([128, 512], FP, tag="s", name="ps_sc")
                        nc.tensor.matmul(ps, hT_in[:, ci * 128:(ci + 1) * 128],
                                         hT_in[:, jb * 512:(jb + 1) * 512],
                                         start=True, stop=False)
                        nc.tensor.matmul(ps, ones_row,
                                         neghalf[:, jb * 512:(jb + 1) * 512],
                                         start=False, stop=True)
                        nc.scalar.activation(sc[:, jb * 512:(jb + 1) * 512], ps,
                                             AF.Copy)
                    vals = work.tile([128, 8], F16, tag="vals", name="vals")
                    nc.vector.max(vals, sc)
                    nc.vector.max_index(idx[:, ci * KNN:(ci + 1) * KNN], vals, sc)

            if conv < 2:
                hT_out = hpool.tile([H, N], FR, tag="hT", name="hT_out")
            else:
                out_row = strips.tile([1, N], FP, tag="outrow", name="out_row")
            emit_scores(0)
            for ib in range(NB):
                # software pipeline: PE computes next block's scores while this
                # block's top-8 + gathers drain on DVE/SWDGE
                if ib + 1 < NB:
                    emit_scores(ib + 1)
                if conv < 2:
                    msgs = work.tile([128, KNN * 512], FP, tag="msgs", name="msgs")
                else:
                    m5 = strips.tile([1, KNN * 512], FP, tag="m5", name="m5")
                for k in range(KNN):
                    pst = psum.tile([128, 512], FP, tag="t", name="pst_xj")
                    for q in range(4):
                        ci = ib * 4 + q
                        # per-q tags: 4 gathers per k would otherwise ping-pong
                        # on a 2-deep buffer, serializing Pool behind PE
                        xj = mlpp.tile([128, H], FP, tag=f"xj{q}", name=f"xj{q}")
                        nc.gpsimd.indirect_dma_start(
                            out=xj, out_offset=None,
                            in_=h_tab[g][conv],
                            in_offset=bass.IndirectOffsetOnAxis(
                                ap=idx[:, ci * KNN + k: ci * KNN + k + 1], axis=0),
                        )
                        nc.tensor.transpose(pst[:, q * 128:(q + 1) * 128], xj, ident)
                    xjT = mlpp.tile([H, 512], FR, tag="xjT", name="xjT")
                    nc.scalar.activation(xjT, pst, AF.Copy)
                    ps1 = psum.tile([128, 512], FP, tag="m1", name="ps1")
                    nc.tensor.matmul(ps1, Bm, xjT, start=True, stop=False)
                    nc.tensor.matmul(ps1, AmB,
                                     hT_in[:, ib * 512:(ib + 1) * 512],
                                     start=False, stop=True)
                    h1 = mlpp.tile([H, 512], FR, tag="h1", name="h1")
                    nc.scalar.activation(h1, ps1, AF.Relu, bias=ba)
                    if conv < 2:
                        ps2 = psum.tile([128, 512], FP, tag="m2", name="ps2")
                        nc.tensor.matmul(ps2, Wb, h1, start=True, stop=True)
                        nc.scalar.activation(msgs[:, k * 512:(k + 1) * 512], ps2,
                                             AF.Relu, bias=bb)
                    else:
                        ps2 = psum.tile([1, 512], FP, tag="m2", name="ps2s")
                        nc.tensor.matmul(ps2, Wb, h1, start=True, stop=True)
                        nc.scalar.activation(m5[:, k * 512:(k + 1) * 512], ps2,
                                             AF.Relu, bias=bb)
                if conv < 2:
                    nc.vector.tensor_reduce(
                        out=hT_out[:, ib * 512:(ib + 1) * 512],
                        in_=msgs.rearrange("p (k i) -> p i k", k=KNN),
                        axis=AX.X, op=ALU.max)
                else:
                    nc.vector.tensor_reduce(
                        out=out_row[:, ib * 512:(ib + 1) * 512],
                        in_=m5.rearrange("p (k i) -> p i k", k=KNN),
                        axis=AX.X, op=ALU.max)
            if conv < 2:
                store_htab(g, conv + 1, hT_out)
                return hT_out
            # sigmoid after max (monotone), then store this graph's 2048 outputs
            sg_row = strips.tile([1, N], FP, tag="sgrow", name="sg_row")
            nc.scalar.activation(sg_row, out_row, AF.Sigmoid)
            dst = out_d.rearrange("(g n) one -> g one n", g=GPC)
            nc.sync.dma_start(dst[g], sg_row)
            return None

        for g in range(GPC):
            hT_cur = hpool.tile([H, N], FR, tag="hT", name="hT_enc")
            for jb in range(NB):
                ps = psum.tile([128, 512], FP, tag="s", name="ps_enc")
                nc.tensor.matmul(ps, w_enc_sb,
                                 xT[:, g * N + jb * 512: g * N + (jb + 1) * 512],
                                 start=True, stop=False)
                nc.tensor.matmul(ps, b_enc_sb, ones_512,
                                 start=False, stop=True)
                nc.scalar.activation(hT_cur[:, jb * 512:(jb + 1) * 512], ps, AF.Copy)
            store_htab(g, 0, hT_cur)
            for conv in range(3):
                hT_cur = edge_conv(g, conv, hT_cur)


def build():
    nc = bass.Bass("TRN2", target_bir_lowering=False, debug=False)
    x_d = nc.dram_tensor("x", [NPC, F_IN], FR, kind="ExternalInput")
    w_aps = {}
    for name, shape in WEIGHT_SPECS.items():
        dt = FR if name in FR_WEIGHTS else FP
        w_aps[name] = nc.dram_tensor(name, list(shape), dt, kind="ExternalInput")[:]
    out_d = nc.dram_tensor("out", [NPC, 1], FP, kind="ExternalOutput")
    with tile.TileContext(nc) as tc:
        emit(tc, x_d[:], out_d[:], w_aps)
    # walrus CoreV3 codegen allows at most 1 sync wait per instruction;
    # split multi-wait instructions via event semaphores (Bacc passes)
    import bass_rust
    bass_rust.move_matmul_waits_to_ldweights(nc.m)
    bass_rust.generate_event_semaphores(nc)
    return nc


def make_in_maps(inputs):
    def f32(a):
        return np.ascontiguousarray(np.asarray(a), dtype=np.float32)
    w = {name: f32(inputs[name]).reshape(shape)
         for name, shape in WEIGHT_SPECS.items()}
    x_full = f32(inputs["x"])
    in_maps = []
    for c in range(CORES):
        m = dict(w)
        m["x"] = np.ascontiguousarray(x_full[c * NPC:(c + 1) * NPC])
        in_maps.append(m)
    return in_maps


def run(inputs, trace=False):
    from concourse.bass_utils import run_bass_kernel_spmd
    nc = build()
    in_maps = make_in_maps(inputs)
    res = run_bass_kernel_spmd(nc, in_maps, list(range(CORES)), trace=trace)
    out = np.concatenate(
        [np.asarray(res.results[c]["out"], dtype=np.float32) for c in range(CORES)],
        axis=0)
    return out, res


def kernel(**inputs):
    out, _ = run(inputs, trace=False)
    return out



# revision 7
# speedup vs baseline: 1.0734x; 1.0734x over previous
"""DynamicEdgeConv GNN (3x EdgeConv + encoder) on 8 TRN2 NeuronCores.

Data-parallel over graphs: 16 graphs of 2048 nodes; 2 graphs per core.
All compute in fp16 (PSUM fp32). Per graph-conv:
  scores(i,j) = h_i.h_j - 0.5||h_j||^2 via PE fp16 matmuls (1024-col blocks,
  neghalf row folded in as a K=1 ones matmul), ACT copy -> sc fp16,
  DVE max8/max_index(u16) -> top-8 neighbor ids.
  Indices are stream-transposed (16x DVE 32x32 blocks) into T2[q', a*128+p]
  and laid into the SWDGE dma_gather wrapped-index format W[i%16, i//16]
  with 2 contiguous SBUF->SBUF DMAs (+2 replicas for Q7 core 1).
  One dma_gather(transpose=True) per 2-chunk node group pulls all K=8
  neighbor feature rows from a node-major fp16 DRAM table and transposes
  them on the fly into feature-major xjT columns - no per-edge PE
  transposes, no per-edge indirect DMAs.
  Edge MLP: [xi, xj-xi]@Wa rewritten as U + xj@B with U = (A-B)^T h + ba
  precomputed per node; the U term enters PSUM via an identity matmul
  whose rhs is a stride-0 broadcast AP (k-replication). Max-aggregation
  over k via DVE tensor_reduce on stride-1 groups of 8.
"""

import numpy as np
from contextlib import ExitStack

import concourse.bass as bass
import concourse.mybir as mybir
from concourse import tile
from concourse import library_config
from concourse import library_overlay
from concourse.masks import make_identity

B_ALL = 16      # graphs total
N = 2048        # nodes per graph
KNN = 8
H = 128
F_IN = 4
CORES = 8
GPC = B_ALL // CORES          # graphs per core
NPC = GPC * N                 # nodes per core
NCH = N // 128                # 16 chunks of 128 nodes per graph

FP = mybir.dt.float32
F16 = mybir.dt.float16
U16 = mybir.dt.uint16
I16 = mybir.dt.int16

AF = mybir.ActivationFunctionType
ALU = mybir.AluOpType
AX = mybir.AxisListType

# gather group gg covers chunk pair (2*SIGMA[gg], 2*SIGMA[gg]+1)
SIGMA = [0, 2, 4, 6, 1, 3, 5, 7]

WEIGHT_SPECS = {
    "W_enc": (F_IN, H), "b_enc": (1, H),
    "W1a": (2 * H, H), "b1a": (H, 1), "W1b": (H, H), "b1b": (H, 1),
    "W2a": (2 * H, H), "b2a": (H, 1), "W2b": (H, H), "b2b": (H, 1),
    "W5a": (2 * H, H), "b5a": (H, 1), "W5b": (H, 1), "b5b": (1, 1),
}

CONV_TAGS = ["1", "2", "5"]


def emit(tc, x, out_d, W):
    nc = tc.nc
    with ExitStack() as ctx:
        consts = ctx.enter_context(tc.tile_pool(name="consts", bufs=1))
        hpool = ctx.enter_context(tc.tile_pool(name="hpool", bufs=4))
        work = ctx.enter_context(tc.tile_pool(name="work", bufs=2))
        upool = ctx.enter_context(tc.tile_pool(name="upool", bufs=2))
        scpool = ctx.enter_context(tc.tile_pool(name="scpool", bufs=2))
        xjpool = ctx.enter_context(tc.tile_pool(name="xjpool", bufs=3))
        mlpp = ctx.enter_context(tc.tile_pool(name="mlpp", bufs=4))
        idxpool = ctx.enter_context(tc.tile_pool(name="idxpool", bufs=2))
        strips = ctx.enter_context(tc.tile_pool(name="strips", bufs=1))
        spsum = ctx.enter_context(tc.tile_pool(name="spsum", bufs=2, space="PSUM"))
        mpsum = ctx.enter_context(tc.tile_pool(name="mpsum", bufs=2, space="PSUM"))
        tpsum = ctx.enter_context(tc.tile_pool(name="tpsum", bufs=2, space="PSUM"))
        hdram = ctx.enter_context(tc.tile_pool(name="hdram", bufs=1, space="DRAM"))

        ident = consts.tile([128, 128], FP, tag="ident", name="ident")
        make_identity(nc, ident)
        id16 = consts.tile([128, 128], F16, tag="id16", name="id16")
        nc.scalar.activation(id16, ident, AF.Copy)
        ones_row = consts.tile([1, 128], F16, tag="ones_row", name="ones_row")
        nc.vector.memset(ones_row, 1.0)
        ones_col = consts.tile([128, 1], F16, tag="ones_col", name="ones_col")
        nc.vector.memset(ones_col, 1.0)

        nc.gpsimd.load_library(library_config.mlp)
        nidx_reg = nc.gpsimd.to_reg(512)

        # ---- weights (fp16 matmul operands, fp32 biases)
        w_enc = consts.tile([F_IN, H], F16, tag="w_enc", name="w_enc")
        nc.gpsimd.dma_start(w_enc, W["W_enc"])
        b_enc = consts.tile([H, 1], FP, tag="b_enc", name="b_enc")
        nc.sync.dma_start(b_enc, W["b_enc"].rearrange("one h -> h one"))

        convW = []
        for t in CONV_TAGS:
            AB = consts.tile([H, 2 * H], FP, tag=f"AB{t}", name=f"AB{t}")
            nc.sync.dma_start(AB.rearrange("h (a j) -> h a j", a=2),
                              W[f"W{t}a"].rearrange("(a h) j -> h a j", a=2))
            Bm = consts.tile([H, H], F16, tag=f"Bm{t}", name=f"Bm{t}")
            nc.scalar.activation(Bm, AB[:, H:2 * H], AF.Copy)
            AmBf = consts.tile([H, H], FP, tag=f"AmBf{t}", name=f"AmBf{t}")
            nc.vector.tensor_sub(AmBf, AB[:, 0:H], AB[:, H:2 * H])
            AmB = consts.tile([H, H], F16, tag=f"AmB{t}", name=f"AmB{t}")
            nc.scalar.activation(AmB, AmBf, AF.Copy)
            ba = consts.tile([H, 1], FP, tag=f"ba{t}", name=f"ba{t}")
            nc.sync.dma_start(ba, W[f"b{t}a"])
            if t != "5":
                Wb = consts.tile([H, H], F16, tag=f"Wb{t}", name=f"Wb{t}")
                bb = consts.tile([H, 1], FP, tag=f"bb{t}", name=f"bb{t}")
            else:
                Wb = consts.tile([H, 1], F16, tag=f"Wb{t}", name=f"Wb{t}")
                bb = consts.tile([1, 1], FP, tag=f"bb{t}", name=f"bb{t}")
            nc.gpsimd.dma_start(Wb, W[f"W{t}b"])
            nc.sync.dma_start(bb, W[f"b{t}b"])
            convW.append((AmB, Bm, ba, Wb, bb))

        # W index tiles (wrapped gather format); partitions 32:128 unused by
        # the ucode but must be initialized for the sim's full-view read.
        wtiles = []
        for par in range(2):
            wt = consts.tile([128, NCH * 64], I16, tag=f"wt{par}", name=f"wt{par}")
            nc.vector.memset(wt, 0)
            wtiles.append(wt)

        # x transposed into SBUF (feature-major) then fp16
        xT = consts.tile([F_IN, NPC], FP, tag="xT", name="xT")
        nc.sync.dma_start(xT, x.rearrange("n f -> f n"))
        xT16 = consts.tile([F_IN, NPC], F16, tag="xT16", name="xT16")
        nc.scalar.activation(xT16, xT, AF.Copy)

        h_nm = [[hdram.tile([N, H], F16, tag=f"hnm_{g}_{c}", name=f"hnm_{g}_{c}")
                 for c in range(3)] for g in range(GPC)]

        def store_hnm(g, layer, hT16):
            dst = h_nm[g][layer].rearrange("(cb q p) f -> cb p q f", q=4, p=128)
            for cb in range(4):
                pst = tpsum.tile([128, 512], F16, tag="t", name="pst_st")
                for q in range(4):
                    col = (cb * 4 + q) * 128
                    nc.tensor.transpose(pst[:, q * 128:(q + 1) * 128],
                                        hT16[:, col:col + 128], id16)
                hsb = work.tile([128, 512], F16, tag="hst", name="hsb")
                nc.scalar.activation(hsb, pst, AF.Copy)
                nc.sync.dma_start(dst[cb], hsb.rearrange("p (q f) -> p q f", q=4))

        def encoder(g):
            hT = hpool.tile([H, N], F16, tag="hT", name="hT_enc")
            for jb in range(2):
                ps = spsum.tile([128, 1024], FP, tag="s", name="ps_enc")
                for q in range(2):
                    col = g * N + jb * 1024 + q * 512
                    nc.tensor.matmul(ps[:, q * 512:(q + 1) * 512], w_enc,
                                     xT16[:, col: col + 512],
                                     start=True, stop=True)
                nc.scalar.activation(hT[:, jb * 1024:(jb + 1) * 1024], ps,
                                     AF.Identity, bias=b_enc)
            store_hnm(g, 0, hT)
            return hT

        def edge_conv(g, conv, hT16):
            AmB, Bm, ba, Wb, bb = convW[conv]
            step = g * 3 + conv

            # squares -> neghalf row (fp16)
            h2 = work.tile([H, N], F16, tag="h2", name="h2")
            nc.scalar.activation(h2, hT16, AF.Square)
            nh = strips.tile([1, N], F16, tag=f"nh{step % 2}", name="nh")
            for jb in range(2):
                ps = spsum.tile([128, 1024], FP, tag="s", name="ps_nh")
                for q in range(2):
                    col = jb * 1024 + q * 512
                    nc.tensor.matmul(ps[0:1, q * 512:(q + 1) * 512], ones_col,
                                     h2[:, col:col + 512],
                                     start=True, stop=True)
                nc.scalar.activation(nh[:, jb * 1024:(jb + 1) * 1024], ps[0:1, :],
                                     AF.Copy, scale=-0.5)

            # U = (A-B)^T h + ba (per node, fp16)
            U = upool.tile([H, N], F16, tag="U", name="U")
            for ub in range(4):
                psm = mpsum.tile([128, 512], FP, tag="m", name="ps_u")
                nc.tensor.matmul(psm, AmB, hT16[:, ub * 512:(ub + 1) * 512],
                                 start=True, stop=True)
                nc.scalar.activation(U[:, ub * 512:(ub + 1) * 512], psm,
                                     AF.Identity, bias=ba)

            # scores + top-8
            idx = idxpool.tile([128, NCH * KNN], U16, tag="idx", name="idx")
            for ci in range(NCH):
                sc = scpool.tile([128, N], F16, tag="sc", name="sc")
                for hb in range(2):
                    ps = spsum.tile([128, 1024], FP, tag="s", name="ps_sc")
                    for q in range(2):
                        col = hb * 1024 + q * 512
                        sl = ps[:, q * 512:(q + 1) * 512]
                        nc.tensor.matmul(sl, hT16[:, ci * 128:(ci + 1) * 128],
                                         hT16[:, col:col + 512],
                                         start=True, stop=False)
                        nc.tensor.matmul(sl, ones_row, nh[:, col:col + 512],
                                         start=False, stop=True)
                    nc.scalar.activation(sc[:, hb * 1024:(hb + 1) * 1024], ps,
                                         AF.Copy)
                vals = work.tile([128, 8], F16, tag="vals", name="vals")
                nc.vector.max(vals, sc)
                nc.vector.max_index(idx[:, ci * KNN:(ci + 1) * KNN], vals, sc)

            # T2[q', a*128+p] = idx[p, 32a+q']
            T2 = idxpool.tile([32, 512], U16, tag="T2", name="T2")
            for a in range(4):
                for b in range(4):
                    nc.vector.transpose(
                        T2[0:32, a * 128 + 32 * b: a * 128 + 32 * b + 32],
                        idx[32 * b:32 * b + 32, 32 * a:32 * a + 32])
            # wrapped index tile: W[q, gg*128+p] = T2[16*(gg//4)+q, (gg%4)*128+p]
            wt = wtiles[step % 2]
            t2i = T2.bitcast(I16)
            nc.sync.dma_start(wt[0:16, 0:512], t2i[0:16, :])
            nc.sync.dma_start(wt[0:16, 512:1024], t2i[16:32, :])
            nc.sync.dma_start(wt[16:32, 0:512], t2i[0:16, :])
            nc.sync.dma_start(wt[16:32, 512:1024], t2i[16:32, :])

            if conv < 2:
                hTo = hpool.tile([H, N], F16, tag="hT", name="hT_out")
            else:
                outrow = strips.tile([1, N], FP, tag=f"outrow{g}", name="outrow")

            for gg in range(8):
                cp = SIGMA[gg]
                # columns (p, ci_lo, k); nodes (2cp+ci_lo)*128 + p
                for m in range(4):
                    xj = xjpool.tile([128, 512], F16, tag="xj", name="xj")
                    nc.gpsimd.dma_gather(
                        out_ap=xj.rearrange("p (a n) -> p a n", a=1),
                        in_ap=h_nm[g][conv][:],
                        idxs_ap=wt[:, gg * 128 + m * 32: gg * 128 + (m + 1) * 32],
                        num_idxs=512,
                        num_idxs_reg=nidx_reg,
                        elem_size=128,
                        transpose=True,
                    )
                    ps1 = mpsum.tile([128, 512], FP, tag="m", name="ps1")
                    nc.tensor.matmul(ps1, Bm, xj, start=True, stop=False)
                    usl = U[:, cp * 256: cp * 256 + 256] \
                        .rearrange("h (c p) -> h p c", c=2)[:, 32 * m:32 * m + 32, :] \
                        .rearrange("h p c -> h p c ()").broadcast_to([H, 32, 2, KNN])
                    nc.tensor.matmul(ps1, id16, usl, start=False, stop=True)
                    h1 = mlpp.tile([H, 512], F16, tag="h1", name="h1")
                    nc.scalar.activation(h1, ps1, AF.Relu)
                    ps2 = mpsum.tile([128, 512], FP, tag="m", name="ps2")
                    if conv < 2:
                        nc.tensor.matmul(ps2, Wb, h1, start=True, stop=True)
                        msgs = mlpp.tile([H, 512], F16, tag="msgs", name="msgs")
                        nc.scalar.activation(msgs, ps2, AF.Relu, bias=bb)
                        nc.vector.tensor_reduce(
                            out=hTo[:, cp * 256: cp * 256 + 256]
                            .rearrange("h (c p) -> h p c", c=2)[:, 32 * m:32 * m + 32, :],
                            in_=msgs.rearrange("h (p c k) -> h p c k", c=2, k=KNN),
                            axis=AX.X, op=ALU.max)
                    else:
                        nc.tensor.matmul(ps2[0:1, :], Wb, h1, start=True, stop=True)
                        m5 = mlpp.tile([1, 512], FP, tag="m5", name="m5")
                        nc.scalar.activation(m5, ps2[0:1, :], AF.Relu, bias=bb)
                        nc.vector.tensor_reduce(
                            out=outrow[:, cp * 256: cp * 256 + 256]
                            .rearrange("h (c p) -> h p c", c=2)[:, 32 * m:32 * m + 32, :],
                            in_=m5.rearrange("h (p c k) -> h p c k", c=2, k=KNN),
                            axis=AX.X, op=ALU.max)

            if conv < 2:
                store_hnm(g, conv + 1, hTo)
                return hTo
            sg = strips.tile([1, N], FP, tag=f"sg{g}", name="sg")
            nc.scalar.activation(sg, outrow, AF.Sigmoid)
            dst = out_d.rearrange("(g n) one -> g one n", g=GPC)
            nc.sync.dma_start(dst[g], sg)
            return None

        hTs = [encoder(g) for g in range(GPC)]
        for conv in range(3):
            for g in range(GPC):
                hTs[g] = edge_conv(g, conv, hTs[g])


def build():
    nc = bass.Bass("TRN2", target_bir_lowering=False, debug=False)
    x_d = nc.dram_tensor("x", [NPC, F_IN], FP, kind="ExternalInput")
    w_aps = {}
    for name, shape in WEIGHT_SPECS.items():
        w_aps[name] = nc.dram_tensor(name, list(shape), FP, kind="ExternalInput")[:]
    out_d = nc.dram_tensor("out", [NPC, 1], FP, kind="ExternalOutput")
    with tile.TileContext(nc) as tc:
        emit(tc, x_d[:], out_d[:], w_aps)
    # walrus CoreV3 codegen allows at most 1 sync wait per instruction;
    # split multi-wait instructions via event semaphores (Bacc passes)
    import bass_rust
    bass_rust.move_matmul_waits_to_ldweights(nc.m)
    bass_rust.generate_event_semaphores(nc)
    # populate .instr bytes for extended-inst ISA subclasses (library
    # reload + dma_gather); raw Bass skips this Bacc pass
    library_overlay.lower_extended_insts(nc)
    return nc


def make_in_maps(inputs):
    def f32(a):
        return np.ascontiguousarray(np.asarray(a), dtype=np.float32)
    w = {name: f32(inputs[name]).reshape(shape)
         for name, shape in WEIGHT_SPECS.items()}
    x_full = f32(inputs["x"])
    in_maps = []
    for c in range(CORES):
        m = dict(w)
        m["x"] = np.ascontiguousarray(x_full[c * NPC:(c + 1) * NPC])
        in_maps.append(m)
    return in_maps


def run(inputs, trace=False):
    from concourse.bass_utils import run_bass_kernel_spmd
    nc = build()
    in_maps = make_in_maps(inputs)
    res = run_bass_kernel_spmd(nc, in_maps, list(range(CORES)), trace=trace)
    out = np.concatenate(
        [np.asarray(res.results[c]["out"], dtype=np.float32) for c in range(CORES)],
        axis=0)
    return out, res


def kernel(**inputs):
    out, _ = run(inputs, trace=False)
    return out


# revision 11
# speedup vs baseline: 2.9987x; 2.7937x over previous
"""DynamicEdgeConv GNN (3x EdgeConv + encoder) on TRN2.

All 16 graphs run on ONE NeuronCore: through this deployment's axon/PJRT
dispatch path, per-core NEFF executions serialize anyway (total device
time is the sum over cores) while host->device input transfer runs at
~50 MB/s with ~1.5 ms per array -- so the winning configuration is one
core (weights shipped once, not replicated 8x) and ONE packed fp16 input
blob (x pre-transposed + weights pre-processed on host).

Per graph-conv (all fp16, PSUM fp32):
  scores(i,j) = h_i.h_j - 0.5||h_j||^2 via PE fp16 matmuls, ACT copy ->
  sc fp16, DVE max8/max_index(u16) -> top-8 neighbor ids.
  Indices are stream-transposed (16x DVE 32x32 blocks) into
  T2[q', a*128+p] and laid into the SWDGE dma_gather wrapped-index format
  W[i%16, i//16] with 2 contiguous SBUF->SBUF DMAs (+2 replicas for Q7
  core 1). dma_gather(transpose=True, 512 idxs/op) pulls neighbor rows
  from a node-major fp16 DRAM table, transposing on the fly into
  feature-major xjT columns -- no per-edge PE transposes or indirect DMAs.
  Edge MLP: [xi, xj-xi]@Wa == U + xj@B with U = (A-B)^T h + ba per node;
  the U term enters PSUM via an identity matmul with a stride-0 broadcast
  rhs. Max over k via DVE tensor_reduce on stride-1 groups of 8.
"""

import numpy as np
from contextlib import ExitStack

import concourse.bass as bass
import concourse.mybir as mybir
from concourse import tile
from concourse import library_config
from concourse import library_overlay
from concourse.masks import make_identity

B_ALL = 16      # graphs total
N = 2048        # nodes per graph
KNN = 8
H = 128
F_IN = 4
CORES = 1
GPC = B_ALL // CORES          # graphs per core
NPC = GPC * N                 # nodes per core
NCH = N // 128                # 16 chunks of 128 nodes per graph

FP = mybir.dt.float32
F16 = mybir.dt.float16
U8 = mybir.dt.uint8
U16 = mybir.dt.uint16
I16 = mybir.dt.int16

AF = mybir.ActivationFunctionType
ALU = mybir.AluOpType
AX = mybir.AxisListType

# gather group gg covers chunk pair (2*SIGMA[gg], 2*SIGMA[gg]+1)
SIGMA = [0, 2, 4, 6, 1, 3, 5, 7]

CONV_TAGS = ["1", "2", "5"]

WEIGHT_SPECS = {
    "W_enc": (F_IN, H), "b_enc": (1, H),
    "W1a": (2 * H, H), "b1a": (H, 1), "W1b": (H, H), "b1b": (H, 1),
    "W2a": (2 * H, H), "b2a": (H, 1), "W2b": (H, H), "b2b": (H, 1),
    "W5a": (2 * H, H), "b5a": (H, 1), "W5b": (H, 1), "b5b": (1, 1),
}


def _blob_layout():
    """Byte offsets of every packed section in the single input blob."""
    off = {}
    pos = 0

    def add(name, nbytes, align=512):
        nonlocal pos
        pos = (pos + align - 1) // align * align
        off[name] = pos
        pos += nbytes

    add("x", F_IN * NPC * 2)                 # f16 [F_IN, NPC] (pre-transposed)
    add("w_enc", F_IN * H * 2)               # f16 [F_IN, H]
    add("b_enc", H * 4)                      # f32 [H, 1]
    for t in CONV_TAGS:
        add(f"AmB{t}", H * H * 2)            # f16 [H, H]
        add(f"Bm{t}", H * H * 2)             # f16 [H, H]
        wb_cols = H if t != "5" else 1
        add(f"Wb{t}", H * wb_cols * 2)       # f16 [H, wb_cols]
        add(f"ba{t}", H * 4)                 # f32 [H, 1]
        add(f"bb{t}", (H if t != "5" else 1) * 4)
    total = (pos + 511) // 512 * 512
    return off, total


BLOB_OFF, BLOB_BYTES = _blob_layout()


def emit(tc, blob, out_d):
    nc = tc.nc

    def bsl(name, nbytes):
        return blob[0:1, BLOB_OFF[name]: BLOB_OFF[name] + nbytes]

    with ExitStack() as ctx:
        consts = ctx.enter_context(tc.tile_pool(name="consts", bufs=1))
        hpool = ctx.enter_context(tc.tile_pool(name="hpool", bufs=4))
        work = ctx.enter_context(tc.tile_pool(name="work", bufs=2))
        upool = ctx.enter_context(tc.tile_pool(name="upool", bufs=2))
        scpool = ctx.enter_context(tc.tile_pool(name="scpool", bufs=2))
        xjpool = ctx.enter_context(tc.tile_pool(name="xjpool", bufs=3))
        mlpp = ctx.enter_context(tc.tile_pool(name="mlpp", bufs=4))
        idxpool = ctx.enter_context(tc.tile_pool(name="idxpool", bufs=2))
        strips = ctx.enter_context(tc.tile_pool(name="strips", bufs=1))
        spsum = ctx.enter_context(tc.tile_pool(name="spsum", bufs=2, space="PSUM"))
        mpsum = ctx.enter_context(tc.tile_pool(name="mpsum", bufs=2, space="PSUM"))
        tpsum = ctx.enter_context(tc.tile_pool(name="tpsum", bufs=2, space="PSUM"))
        hdram = ctx.enter_context(tc.tile_pool(name="hdram", bufs=1, space="DRAM"))

        ident = consts.tile([128, 128], FP, tag="ident", name="ident")
        make_identity(nc, ident)
        id16 = consts.tile([128, 128], F16, tag="id16", name="id16")
        nc.scalar.activation(id16, ident, AF.Copy)
        ones_row = consts.tile([1, 128], F16, tag="ones_row", name="ones_row")
        nc.vector.memset(ones_row, 1.0)
        ones_col = consts.tile([128, 1], F16, tag="ones_col", name="ones_col")
        nc.vector.memset(ones_col, 1.0)

        nc.gpsimd.load_library(library_config.mlp)
        nidx_reg = nc.gpsimd.to_reg(512)

        # ---- unpack weights from the blob
        w_enc = consts.tile([F_IN, H], F16, tag="w_enc", name="w_enc")
        nc.sync.dma_start(w_enc, bsl("w_enc", F_IN * H * 2).bitcast(F16)
                          .rearrange("one (f h) -> (one f) h", f=F_IN))
        b_enc = consts.tile([H, 1], FP, tag="b_enc", name="b_enc")
        nc.sync.dma_start(b_enc, bsl("b_enc", H * 4).bitcast(FP)
                          .rearrange("one (h z) -> (one h) z", z=1))

        convW = []
        for t in CONV_TAGS:
            AmB = consts.tile([H, H], F16, tag=f"AmB{t}", name=f"AmB{t}")
            nc.sync.dma_start(AmB, bsl(f"AmB{t}", H * H * 2).bitcast(F16)
                              .rearrange("one (h j) -> (one h) j", h=H))
            Bm = consts.tile([H, H], F16, tag=f"Bm{t}", name=f"Bm{t}")
            nc.sync.dma_start(Bm, bsl(f"Bm{t}", H * H * 2).bitcast(F16)
                              .rearrange("one (h j) -> (one h) j", h=H))
            wb_cols = H if t != "5" else 1
            Wb = consts.tile([H, wb_cols], F16, tag=f"Wb{t}", name=f"Wb{t}")
            nc.sync.dma_start(Wb, bsl(f"Wb{t}", H * wb_cols * 2).bitcast(F16)
                              .rearrange("one (h j) -> (one h) j", h=H))
            ba = consts.tile([H, 1], FP, tag=f"ba{t}", name=f"ba{t}")
            nc.sync.dma_start(ba, bsl(f"ba{t}", H * 4).bitcast(FP)
                              .rearrange("one (h z) -> (one h) z", z=1))
            nbb = H if t != "5" else 1
            bb = consts.tile([nbb, 1], FP, tag=f"bb{t}", name=f"bb{t}")
            nc.sync.dma_start(bb, bsl(f"bb{t}", nbb * 4).bitcast(FP)
                              .rearrange("one (h z) -> (one h) z", z=1))
            convW.append((AmB, Bm, ba, Wb, bb))

        # W index tiles (wrapped gather format); partitions 32:128 unused by
        # the ucode but must be initialized for the sim's full-view read.
        wtiles = []
        for par in range(2):
            wt = consts.tile([128, NCH * 64], I16, tag=f"wt{par}", name=f"wt{par}")
            nc.vector.memset(wt, 0)
            wtiles.append(wt)

        # x, already feature-major fp16 on host
        xT16 = consts.tile([F_IN, NPC], F16, tag="xT16", name="xT16")
        nc.sync.dma_start(xT16, bsl("x", F_IN * NPC * 2).bitcast(F16)
                          .rearrange("one (f n) -> (one f) n", f=F_IN))

        h_nm = [[hdram.tile([N, H], F16, tag=f"hnm_{g}_{c}", name=f"hnm_{g}_{c}")
                 for c in range(3)] for g in range(GPC)]

        def store_hnm(g, layer, hT16):
            dst = h_nm[g][layer].rearrange("(cb q p) f -> cb p q f", q=4, p=128)
            for cb in range(4):
                pst = tpsum.tile([128, 512], F16, tag="t", name="pst_st")
                for q in range(4):
                    col = (cb * 4 + q) * 128
                    nc.tensor.transpose(pst[:, q * 128:(q + 1) * 128],
                                        hT16[:, col:col + 128], id16)
                hsb = work.tile([128, 512], F16, tag="hst", name="hsb")
                nc.scalar.activation(hsb, pst, AF.Copy)
                nc.sync.dma_start(dst[cb], hsb.rearrange("p (q f) -> p q f", q=4))

        def encoder(g):
            hT = hpool.tile([H, N], F16, tag="hT", name="hT_enc")
            for jb in range(2):
                ps = spsum.tile([128, 1024], FP, tag="s", name="ps_enc")
                for q in range(2):
                    col = g * N + jb * 1024 + q * 512
                    nc.tensor.matmul(ps[:, q * 512:(q + 1) * 512], w_enc,
                                     xT16[:, col: col + 512],
                                     start=True, stop=True)
                nc.scalar.activation(hT[:, jb * 1024:(jb + 1) * 1024], ps,
                                     AF.Identity, bias=b_enc)
            store_hnm(g, 0, hT)
            return hT

        def edge_conv(g, conv, hT16):
            AmB, Bm, ba, Wb, bb = convW[conv]
            step = g * 3 + conv

            # squares -> neghalf row (fp16)
            h2 = work.tile([H, N], F16, tag="h2", name="h2")
            nc.scalar.activation(h2, hT16, AF.Square)
            nh = strips.tile([1, N], F16, tag=f"nh{step % 2}", name="nh")
            for jb in range(2):
                ps = spsum.tile([128, 1024], FP, tag="s", name="ps_nh")
                for q in range(2):
                    col = jb * 1024 + q * 512
                    nc.tensor.matmul(ps[0:1, q * 512:(q + 1) * 512], ones_col,
                                     h2[:, col:col + 512],
                                     start=True, stop=True)
                nc.scalar.activation(nh[:, jb * 1024:(jb + 1) * 1024], ps[0:1, :],
                                     AF.Copy, scale=-0.5)

            # U = (A-B)^T h + ba (per node, fp16)
            U = upool.tile([H, N], F16, tag="U", name="U")
            for ub in range(4):
                psm = mpsum.tile([128, 512], FP, tag="m", name="ps_u")
                nc.tensor.matmul(psm, AmB, hT16[:, ub * 512:(ub + 1) * 512],
                                 start=True, stop=True)
                nc.scalar.activation(U[:, ub * 512:(ub + 1) * 512], psm,
                                     AF.Identity, bias=ba)

            # scores + top-8
            idx = idxpool.tile([128, NCH * KNN], U16, tag="idx", name="idx")
            for ci in range(NCH):
                sc = scpool.tile([128, N], F16, tag="sc", name="sc")
                for hb in range(2):
                    ps = spsum.tile([128, 1024], FP, tag="s", name="ps_sc")
                    for q in range(2):
                        col = hb * 1024 + q * 512
                        sl = ps[:, q * 512:(q + 1) * 512]
                        nc.tensor.matmul(sl, hT16[:, ci * 128:(ci + 1) * 128],
                                         hT16[:, col:col + 512],
                                         start=True, stop=False)
                        nc.tensor.matmul(sl, ones_row, nh[:, col:col + 512],
                                         start=False, stop=True)
                    nc.scalar.activation(sc[:, hb * 1024:(hb + 1) * 1024], ps,
                                         AF.Copy)
                vals = work.tile([128, 8], F16, tag="vals", name="vals")
                nc.vector.max(vals, sc)
                nc.vector.max_index(idx[:, ci * KNN:(ci + 1) * KNN], vals, sc)

            # T2[q', a*128+p] = idx[p, 32a+q']
            T2 = idxpool.tile([32, 512], U16, tag="T2", name="T2")
            for a in range(4):
                for b in range(4):
                    nc.vector.transpose(
                        T2[0:32, a * 128 + 32 * b: a * 128 + 32 * b + 32],
                        idx[32 * b:32 * b + 32, 32 * a:32 * a + 32])
            # wrapped index tile: W[q, gg*128+p] = T2[16*(gg//4)+q, (gg%4)*128+p]
            wt = wtiles[step % 2]
            t2i = T2.bitcast(I16)
            nc.sync.dma_start(wt[0:16, 0:512], t2i[0:16, :])
            nc.sync.dma_start(wt[0:16, 512:1024], t2i[16:32, :])
            nc.sync.dma_start(wt[16:32, 0:512], t2i[0:16, :])
            nc.sync.dma_start(wt[16:32, 512:1024], t2i[16:32, :])

            if conv < 2:
                hTo = hpool.tile([H, N], F16, tag="hT", name="hT_out")
            else:
                outrow = scpool.tile([1, N], FP, tag="outrow", name="outrow")

            for gg in range(8):
                cp = SIGMA[gg]
                # columns (p, ci_lo, k); nodes (2cp+ci_lo)*128 + p
                for m in range(4):
                    xj = xjpool.tile([128, 512], F16, tag="xj", name="xj")
                    nc.gpsimd.dma_gather(
                        out_ap=xj.rearrange("p (a n) -> p a n", a=1),
                        in_ap=h_nm[g][conv][:],
                        idxs_ap=wt[:, gg * 128 + m * 32: gg * 128 + (m + 1) * 32],
                        num_idxs=512,
                        num_idxs_reg=nidx_reg,
                        elem_size=128,
                        transpose=True,
                    )
                    ps1 = mpsum.tile([128, 512], FP, tag="m", name="ps1")
                    nc.tensor.matmul(ps1, Bm, xj, start=True, stop=False)
                    usl = U[:, cp * 256: cp * 256 + 256] \
                        .rearrange("h (c p) -> h p c", c=2)[:, 32 * m:32 * m + 32, :] \
                        .rearrange("h p c -> h p c ()").broadcast_to([H, 32, 2, KNN])
                    nc.tensor.matmul(ps1, id16, usl, start=False, stop=True)
                    h1 = mlpp.tile([H, 512], F16, tag="h1", name="h1")
                    nc.scalar.activation(h1, ps1, AF.Relu)
                    ps2 = mpsum.tile([128, 512], FP, tag="m", name="ps2")
                    if conv < 2:
                        nc.tensor.matmul(ps2, Wb, h1, start=True, stop=True)
                        msgs = mlpp.tile([H, 512], F16, tag="msgs", name="msgs")
                        nc.scalar.activation(msgs, ps2, AF.Relu, bias=bb)
                        nc.vector.tensor_reduce(
                            out=hTo[:, cp * 256: cp * 256 + 256]
                            .rearrange("h (c p) -> h p c", c=2)[:, 32 * m:32 * m + 32, :],
                            in_=msgs.rearrange("h (p c k) -> h p c k", c=2, k=KNN),
                            axis=AX.X, op=ALU.max)
                    else:
                        nc.tensor.matmul(ps2[0:1, :], Wb, h1, start=True, stop=True)
                        m5 = mlpp.tile([1, 512], FP, tag="m5", name="m5")
                        nc.scalar.activation(m5, ps2[0:1, :], AF.Relu, bias=bb)
                        nc.vector.tensor_reduce(
                            out=outrow[:, cp * 256: cp * 256 + 256]
                            .rearrange("h (c p) -> h p c", c=2)[:, 32 * m:32 * m + 32, :],
                            in_=m5.rearrange("h (p c k) -> h p c k", c=2, k=KNN),
                            axis=AX.X, op=ALU.max)

            if conv < 2:
                store_hnm(g, conv + 1, hTo)
                return hTo
            sg = scpool.tile([1, N], FP, tag="sg", name="sg")
            nc.scalar.activation(sg, outrow, AF.Sigmoid)
            dst = out_d.rearrange("(g n) one -> g one n", g=GPC)
            nc.sync.dma_start(dst[g], sg)
            return None

        # process graphs in pairs so two graphs' stages overlap
        for pair in range(GPC // 2):
            gs = (2 * pair, 2 * pair + 1)
            hTs = {g: encoder(g) for g in gs}
            for conv in range(3):
                for g in gs:
                    hTs[g] = edge_conv(g, conv, hTs[g])


def build():
    nc = bass.Bass("TRN2", target_bir_lowering=False, debug=False)
    blob_d = nc.dram_tensor("blob", [1, BLOB_BYTES], U8, kind="ExternalInput")
    out_d = nc.dram_tensor("out", [NPC, 1], FP, kind="ExternalOutput")
    with tile.TileContext(nc) as tc:
        emit(tc, blob_d[:], out_d[:])
    # walrus CoreV3 codegen allows at most 1 sync wait per instruction;
    # split multi-wait instructions via event semaphores (Bacc passes)
    import bass_rust
    bass_rust.move_matmul_waits_to_ldweights(nc.m)
    bass_rust.generate_event_semaphores(nc)
    # populate .instr bytes for extended-inst ISA subclasses (library
    # reload + dma_gather); raw Bass skips this Bacc pass
    library_overlay.lower_extended_insts(nc)
    return nc


def pack_blob(inputs, core):
    def f32(a):
        return np.ascontiguousarray(np.asarray(a), dtype=np.float32)
    blob = np.zeros(BLOB_BYTES, np.uint8)

    def put(name, arr):
        b = np.ascontiguousarray(arr).tobytes()
        blob[BLOB_OFF[name]: BLOB_OFF[name] + len(b)] = np.frombuffer(b, np.uint8)

    x = f32(inputs["x"])[core * NPC:(core + 1) * NPC]          # [NPC, F_IN]
    put("x", np.ascontiguousarray(x.T).astype(np.float16))     # [F_IN, NPC]
    put("w_enc", f32(inputs["W_enc"]).reshape(F_IN, H).astype(np.float16))
    put("b_enc", f32(inputs["b_enc"]).reshape(H))
    for t in CONV_TAGS:
        wa = f32(inputs[f"W{t}a"]).reshape(2 * H, H)
        put(f"AmB{t}", (wa[0:H] - wa[H:2 * H]).astype(np.float16))
        put(f"Bm{t}", wa[H:2 * H].astype(np.float16))
        put(f"Wb{t}", f32(inputs[f"W{t}b"]).reshape(H, -1).astype(np.float16))
        put(f"ba{t}", f32(inputs[f"b{t}a"]).reshape(H))
        put(f"bb{t}", f32(inputs[f"b{t}b"]).reshape(-1))
    return blob.reshape(1, BLOB_BYTES)


def make_in_maps(inputs):
    return [{"blob": pack_blob(inputs, c)} for c in range(CORES)]


def run(inputs, trace=False):
    from concourse.bass_utils import run_bass_kernel_spmd
    nc = build()
    in_maps = make_in_maps(inputs)
    res = run_bass_kernel_spmd(nc, in_maps, list(range(CORES)), trace=trace)
    out = np.concatenate(
        [np.asarray(res.results[c]["out"], dtype=np.float32) for c in range(CORES)],
        axis=0)
    return out, res


def kernel(**inputs):
    out, _ = run(inputs, trace=False)
    return out


# revision 12
# speedup vs baseline: 3.1517x; 1.0510x over previous
"""DynamicEdgeConv GNN (3x EdgeConv + encoder) on TRN2.

All 16 graphs run on ONE NeuronCore: through this deployment's axon/PJRT
dispatch path, per-core NEFF executions serialize anyway (total device
time is the sum over cores) while host->device input transfer runs at
~50 MB/s with ~1.5 ms per array -- so the winning configuration is one
core (weights shipped once, not replicated 8x) and ONE packed fp16 input
blob (x pre-transposed + weights pre-processed on host).

Per graph-conv (all fp16, PSUM fp32):
  scores(i,j) = h_i.h_j - 0.5||h_j||^2 via PE fp16 matmuls, ACT copy ->
  sc fp16, DVE max8/max_index(u16) -> top-8 neighbor ids.
  Indices are stream-transposed (16x DVE 32x32 blocks) into
  T2[q', a*128+p] and laid into the SWDGE dma_gather wrapped-index format
  W[i%16, i//16] with 2 contiguous SBUF->SBUF DMAs (+2 replicas for Q7
  core 1). dma_gather(transpose=True, 512 idxs/op) pulls neighbor rows
  from a node-major fp16 DRAM table, transposing on the fly into
  feature-major xjT columns -- no per-edge PE transposes or indirect DMAs.
  Edge MLP: [xi, xj-xi]@Wa == U + xj@B with U = (A-B)^T h + ba per node;
  the U term enters PSUM via an identity matmul with a stride-0 broadcast
  rhs. Max over k via DVE tensor_reduce on stride-1 groups of 8.
"""

import numpy as np
from contextlib import ExitStack

import concourse.bass as bass
import concourse.mybir as mybir
from concourse import tile
from concourse import library_config
from concourse import library_overlay
from concourse.masks import make_identity

B_ALL = 16      # graphs total
N = 2048        # nodes per graph
KNN = 8
H = 128
F_IN = 4
CORES = 1
GPC = B_ALL // CORES          # graphs per core
NPC = GPC * N                 # nodes per core
NCH = N // 128                # 16 chunks of 128 nodes per graph

FP = mybir.dt.float32
F16 = mybir.dt.float16
U8 = mybir.dt.uint8
U16 = mybir.dt.uint16
I16 = mybir.dt.int16

AF = mybir.ActivationFunctionType
ALU = mybir.AluOpType
AX = mybir.AxisListType

# gather group gg covers chunk pair (2*SIGMA[gg], 2*SIGMA[gg]+1)
SIGMA = [0, 2, 4, 6, 1, 3, 5, 7]

CONV_TAGS = ["1", "2", "5"]

WEIGHT_SPECS = {
    "W_enc": (F_IN, H), "b_enc": (1, H),
    "W1a": (2 * H, H), "b1a": (H, 1), "W1b": (H, H), "b1b": (H, 1),
    "W2a": (2 * H, H), "b2a": (H, 1), "W2b": (H, H), "b2b": (H, 1),
    "W5a": (2 * H, H), "b5a": (H, 1), "W5b": (H, 1), "b5b": (1, 1),
}


def _blob_layout():
    """Byte offsets of every packed section in the single input blob."""
    off = {}
    pos = 0

    def add(name, nbytes, align=512):
        nonlocal pos
        pos = (pos + align - 1) // align * align
        off[name] = pos
        pos += nbytes

    add("x", F_IN * NPC * 2)                 # f16 [F_IN, NPC] (pre-transposed)
    add("w_enc", F_IN * H)                   # i8 [F_IN, H]
    add("w_enc_s", F_IN * 4)                 # f32 [F_IN, 1] dequant scale
    add("b_enc", H * 4)                      # f32 [H, 1]
    for t in CONV_TAGS:
        add(f"AmB{t}", H * H)                # i8 [H, H]
        add(f"Bm{t}", H * H)                 # i8 [H, H]
        wb_cols = H if t != "5" else 1
        add(f"Wb{t}", H * wb_cols)           # i8 [H, wb_cols]
        add(f"ws{t}", 3 * H * 4)             # f32 scales [3][H, 1] (AmB, Bm, Wb)
        add(f"ba{t}", H * 4)                 # f32 [H, 1]
        add(f"bb{t}", (H if t != "5" else 1) * 4)
    total = (pos + 511) // 512 * 512
    return off, total


BLOB_OFF, BLOB_BYTES = _blob_layout()


def emit(tc, blob, out_d):
    nc = tc.nc

    def bsl(name, nbytes):
        return blob[0:1, BLOB_OFF[name]: BLOB_OFF[name] + nbytes]

    with ExitStack() as ctx:
        consts = ctx.enter_context(tc.tile_pool(name="consts", bufs=1))
        hpool = ctx.enter_context(tc.tile_pool(name="hpool", bufs=4))
        work = ctx.enter_context(tc.tile_pool(name="work", bufs=2))
        upool = ctx.enter_context(tc.tile_pool(name="upool", bufs=2))
        scpool = ctx.enter_context(tc.tile_pool(name="scpool", bufs=2))
        xjpool = ctx.enter_context(tc.tile_pool(name="xjpool", bufs=3))
        mlpp = ctx.enter_context(tc.tile_pool(name="mlpp", bufs=4))
        idxpool = ctx.enter_context(tc.tile_pool(name="idxpool", bufs=2))
        strips = ctx.enter_context(tc.tile_pool(name="strips", bufs=1))
        spsum = ctx.enter_context(tc.tile_pool(name="spsum", bufs=2, space="PSUM"))
        mpsum = ctx.enter_context(tc.tile_pool(name="mpsum", bufs=2, space="PSUM"))
        tpsum = ctx.enter_context(tc.tile_pool(name="tpsum", bufs=2, space="PSUM"))
        hdram = ctx.enter_context(tc.tile_pool(name="hdram", bufs=1, space="DRAM"))

        ident = consts.tile([128, 128], FP, tag="ident", name="ident")
        make_identity(nc, ident)
        id16 = consts.tile([128, 128], F16, tag="id16", name="id16")
        nc.scalar.activation(id16, ident, AF.Copy)
        ones_row = consts.tile([1, 128], F16, tag="ones_row", name="ones_row")
        nc.vector.memset(ones_row, 1.0)
        ones_col = consts.tile([128, 1], F16, tag="ones_col", name="ones_col")
        nc.vector.memset(ones_col, 1.0)

        nc.gpsimd.load_library(library_config.mlp)
        nidx_reg = nc.gpsimd.to_reg(512)

        # ---- unpack weights from the blob (int8 + per-tensor scale rows)
        I8 = mybir.dt.int8

        def dequant(name_q, name_s, s_off, rows, cols, tagp):
            qt = consts.tile([rows, cols], I8, tag=f"{tagp}_q", name=f"{tagp}_q")
            nc.sync.dma_start(qt, bsl(name_q, rows * cols).bitcast(I8)
                              .rearrange("one (h j) -> (one h) j", h=rows))
            st = consts.tile([rows, 1], FP, tag=f"{tagp}_s", name=f"{tagp}_s")
            nc.sync.dma_start(st, bsl(name_s, (s_off + 1) * rows * 4).bitcast(FP)
                              [:, s_off * rows:(s_off + 1) * rows]
                              .rearrange("one (h z) -> (one h) z", z=1))
            ft = consts.tile([rows, cols], F16, tag=f"{tagp}_f", name=f"{tagp}_f")
            nc.vector.tensor_scalar_mul(ft, qt, st)
            return ft

        w_enc = dequant("w_enc", "w_enc_s", 0, F_IN, H, "w_enc")
        b_enc = consts.tile([H, 1], FP, tag="b_enc", name="b_enc")
        nc.sync.dma_start(b_enc, bsl("b_enc", H * 4).bitcast(FP)
                          .rearrange("one (h z) -> (one h) z", z=1))

        convW = []
        for t in CONV_TAGS:
            AmB = dequant(f"AmB{t}", f"ws{t}", 0, H, H, f"AmB{t}")
            Bm = dequant(f"Bm{t}", f"ws{t}", 1, H, H, f"Bm{t}")
            wb_cols = H if t != "5" else 1
            Wb = dequant(f"Wb{t}", f"ws{t}", 2, H, wb_cols, f"Wb{t}")
            ba = consts.tile([H, 1], FP, tag=f"ba{t}", name=f"ba{t}")
            nc.sync.dma_start(ba, bsl(f"ba{t}", H * 4).bitcast(FP)
                              .rearrange("one (h z) -> (one h) z", z=1))
            nbb = H if t != "5" else 1
            bb = consts.tile([nbb, 1], FP, tag=f"bb{t}", name=f"bb{t}")
            nc.sync.dma_start(bb, bsl(f"bb{t}", nbb * 4).bitcast(FP)
                              .rearrange("one (h z) -> (one h) z", z=1))
            convW.append((AmB, Bm, ba, Wb, bb))

        # W index tiles (wrapped gather format); partitions 32:128 unused by
        # the ucode but must be initialized for the sim's full-view read.
        wtiles = []
        for par in range(2):
            wt = consts.tile([128, NCH * 64], I16, tag=f"wt{par}", name=f"wt{par}")
            nc.vector.memset(wt, 0)
            wtiles.append(wt)

        # x, already feature-major fp16 on host
        xT16 = consts.tile([F_IN, NPC], F16, tag="xT16", name="xT16")
        nc.sync.dma_start(xT16, bsl("x", F_IN * NPC * 2).bitcast(F16)
                          .rearrange("one (f n) -> (one f) n", f=F_IN))

        h_nm = [[hdram.tile([N, H], F16, tag=f"hnm_{g}_{c}", name=f"hnm_{g}_{c}")
                 for c in range(3)] for g in range(GPC)]

        def store_hnm(g, layer, hT16):
            dst = h_nm[g][layer].rearrange("(cb q p) f -> cb p q f", q=4, p=128)
            for cb in range(4):
                pst = tpsum.tile([128, 512], F16, tag="t", name="pst_st")
                for q in range(4):
                    col = (cb * 4 + q) * 128
                    nc.tensor.transpose(pst[:, q * 128:(q + 1) * 128],
                                        hT16[:, col:col + 128], id16)
                hsb = work.tile([128, 512], F16, tag="hst", name="hsb")
                nc.scalar.activation(hsb, pst, AF.Copy)
                nc.sync.dma_start(dst[cb], hsb.rearrange("p (q f) -> p q f", q=4))

        def encoder(g):
            hT = hpool.tile([H, N], F16, tag="hT", name="hT_enc")
            for jb in range(2):
                ps = spsum.tile([128, 1024], FP, tag="s", name="ps_enc")
                for q in range(2):
                    col = g * N + jb * 1024 + q * 512
                    nc.tensor.matmul(ps[:, q * 512:(q + 1) * 512], w_enc,
                                     xT16[:, col: col + 512],
                                     start=True, stop=True)
                nc.scalar.activation(hT[:, jb * 1024:(jb + 1) * 1024], ps,
                                     AF.Identity, bias=b_enc)
            store_hnm(g, 0, hT)
            return hT

        def edge_conv(g, conv, hT16):
            AmB, Bm, ba, Wb, bb = convW[conv]
            step = g * 3 + conv

            # squares -> neghalf row (fp16)
            h2 = work.tile([H, N], F16, tag="h2", name="h2")
            nc.scalar.activation(h2, hT16, AF.Square)
            nh = strips.tile([1, N], F16, tag=f"nh{step % 2}", name="nh")
            for jb in range(2):
                ps = spsum.tile([128, 1024], FP, tag="s", name="ps_nh")
                for q in range(2):
                    col = jb * 1024 + q * 512
                    nc.tensor.matmul(ps[0:1, q * 512:(q + 1) * 512], ones_col,
                                     h2[:, col:col + 512],
                                     start=True, stop=True)
                nc.scalar.activation(nh[:, jb * 1024:(jb + 1) * 1024], ps[0:1, :],
                                     AF.Copy, scale=-0.5)

            # U = (A-B)^T h + ba (per node, fp16)
            U = upool.tile([H, N], F16, tag="U", name="U")
            for ub in range(4):
                psm = mpsum.tile([128, 512], FP, tag="m", name="ps_u")
                nc.tensor.matmul(psm, AmB, hT16[:, ub * 512:(ub + 1) * 512],
                                 start=True, stop=True)
                nc.scalar.activation(U[:, ub * 512:(ub + 1) * 512], psm,
                                     AF.Identity, bias=ba)

            # scores + top-8
            idx = idxpool.tile([128, NCH * KNN], U16, tag="idx", name="idx")
            for ci in range(NCH):
                sc = scpool.tile([128, N], F16, tag="sc", name="sc")
                for hb in range(2):
                    ps = spsum.tile([128, 1024], FP, tag="s", name="ps_sc")
                    for q in range(2):
                        col = hb * 1024 + q * 512
                        sl = ps[:, q * 512:(q + 1) * 512]
                        nc.tensor.matmul(sl, hT16[:, ci * 128:(ci + 1) * 128],
                                         hT16[:, col:col + 512],
                                         start=True, stop=False)
                        nc.tensor.matmul(sl, ones_row, nh[:, col:col + 512],
                                         start=False, stop=True)
                    nc.scalar.activation(sc[:, hb * 1024:(hb + 1) * 1024], ps,
                                         AF.Copy)
                vals = work.tile([128, 8], F16, tag="vals", name="vals")
                nc.vector.max(vals, sc)
                nc.vector.max_index(idx[:, ci * KNN:(ci + 1) * KNN], vals, sc)

            # T2[q', a*128+p] = idx[p, 32a+q']
            T2 = idxpool.tile([32, 512], U16, tag="T2", name="T2")
            for a in range(4):
                for b in range(4):
                    nc.vector.transpose(
                        T2[0:32, a * 128 + 32 * b: a * 128 + 32 * b + 32],
                        idx[32 * b:32 * b + 32, 32 * a:32 * a + 32])
            # wrapped index tile: W[q, gg*128+p] = T2[16*(gg//4)+q, (gg%4)*128+p]
            wt = wtiles[step % 2]
            t2i = T2.bitcast(I16)
            nc.sync.dma_start(wt[0:16, 0:512], t2i[0:16, :])
            nc.sync.dma_start(wt[0:16, 512:1024], t2i[16:32, :])
            nc.sync.dma_start(wt[16:32, 0:512], t2i[0:16, :])
            nc.sync.dma_start(wt[16:32, 512:1024], t2i[16:32, :])

            if conv < 2:
                hTo = hpool.tile([H, N], F16, tag="hT", name="hT_out")
            else:
                outrow = scpool.tile([1, N], FP, tag="outrow", name="outrow")

            for gg in range(8):
                cp = SIGMA[gg]
                # columns (p, ci_lo, k); nodes (2cp+ci_lo)*128 + p
                for m in range(4):
                    xj = xjpool.tile([128, 512], F16, tag="xj", name="xj")
                    nc.gpsimd.dma_gather(
                        out_ap=xj.rearrange("p (a n) -> p a n", a=1),
                        in_ap=h_nm[g][conv][:],
                        idxs_ap=wt[:, gg * 128 + m * 32: gg * 128 + (m + 1) * 32],
                        num_idxs=512,
                        num_idxs_reg=nidx_reg,
                        elem_size=128,
                        transpose=True,
                    )
                    ps1 = mpsum.tile([128, 512], FP, tag="m", name="ps1")
                    nc.tensor.matmul(ps1, Bm, xj, start=True, stop=False)
                    usl = U[:, cp * 256: cp * 256 + 256] \
                        .rearrange("h (c p) -> h p c", c=2)[:, 32 * m:32 * m + 32, :] \
                        .rearrange("h p c -> h p c ()").broadcast_to([H, 32, 2, KNN])
                    nc.tensor.matmul(ps1, id16, usl, start=False, stop=True)
                    h1 = mlpp.tile([H, 512], F16, tag="h1", name="h1")
                    nc.scalar.activation(h1, ps1, AF.Relu)
                    ps2 = mpsum.tile([128, 512], FP, tag="m", name="ps2")
                    if conv < 2:
                        nc.tensor.matmul(ps2, Wb, h1, start=True, stop=True)
                        msgs = mlpp.tile([H, 512], F16, tag="msgs", name="msgs")
                        nc.scalar.activation(msgs, ps2, AF.Relu, bias=bb)
                        nc.vector.tensor_reduce(
                            out=hTo[:, cp * 256: cp * 256 + 256]
                            .rearrange("h (c p) -> h p c", c=2)[:, 32 * m:32 * m + 32, :],
                            in_=msgs.rearrange("h (p c k) -> h p c k", c=2, k=KNN),
                            axis=AX.X, op=ALU.max)
                    else:
                        nc.tensor.matmul(ps2[0:1, :], Wb, h1, start=True, stop=True)
                        m5 = mlpp.tile([1, 512], FP, tag="m5", name="m5")
                        nc.scalar.activation(m5, ps2[0:1, :], AF.Relu, bias=bb)
                        nc.vector.tensor_reduce(
                            out=outrow[:, cp * 256: cp * 256 + 256]
                            .rearrange("h (c p) -> h p c", c=2)[:, 32 * m:32 * m + 32, :],
                            in_=m5.rearrange("h (p c k) -> h p c k", c=2, k=KNN),
                            axis=AX.X, op=ALU.max)

            if conv < 2:
                store_hnm(g, conv + 1, hTo)
                return hTo
            sg = scpool.tile([1, N], F16, tag="sg", name="sg")
            nc.scalar.activation(sg, outrow, AF.Sigmoid)
            dst = out_d.rearrange("(g n) one -> g one n", g=GPC)
            nc.sync.dma_start(dst[g], sg)
            return None

        # process graphs in pairs so two graphs' stages overlap
        for pair in range(GPC // 2):
            gs = (2 * pair, 2 * pair + 1)
            hTs = {g: encoder(g) for g in gs}
            for conv in range(3):
                for g in gs:
                    hTs[g] = edge_conv(g, conv, hTs[g])


def build():
    nc = bass.Bass("TRN2", target_bir_lowering=False, debug=False)
    blob_d = nc.dram_tensor("blob", [1, BLOB_BYTES], U8, kind="ExternalInput")
    out_d = nc.dram_tensor("out", [NPC, 1], F16, kind="ExternalOutput")
    with tile.TileContext(nc) as tc:
        emit(tc, blob_d[:], out_d[:])
    # walrus CoreV3 codegen allows at most 1 sync wait per instruction;
    # split multi-wait instructions via event semaphores (Bacc passes)
    import bass_rust
    bass_rust.move_matmul_waits_to_ldweights(nc.m)
    bass_rust.generate_event_semaphores(nc)
    # populate .instr bytes for extended-inst ISA subclasses (library
    # reload + dma_gather); raw Bass skips this Bacc pass
    library_overlay.lower_extended_insts(nc)
    return nc


def pack_blob(inputs, core):
    def f32(a):
        return np.ascontiguousarray(np.asarray(a), dtype=np.float32)
    blob = np.zeros(BLOB_BYTES, np.uint8)

    def put(name, arr):
        b = np.ascontiguousarray(arr).tobytes()
        blob[BLOB_OFF[name]: BLOB_OFF[name] + len(b)] = np.frombuffer(b, np.uint8)

    def quant(w):
        s = np.abs(w).max(axis=1, keepdims=True) / 127.0
        s = np.maximum(s, 1e-12)
        q = np.clip(np.round(w / s), -127, 127).astype(np.int8)
        return q, s.reshape(-1).astype(np.float32)

    x = f32(inputs["x"])[core * NPC:(core + 1) * NPC]          # [NPC, F_IN]
    put("x", np.ascontiguousarray(x.T).astype(np.float16))     # [F_IN, NPC]
    wq, wsc = quant(f32(inputs["W_enc"]).reshape(F_IN, H))
    put("w_enc", wq)
    put("w_enc_s", wsc)
    put("b_enc", f32(inputs["b_enc"]).reshape(H))
    for t in CONV_TAGS:
        wa = f32(inputs[f"W{t}a"]).reshape(2 * H, H)
        q0, s0 = quant(wa[0:H] - wa[H:2 * H])
        q1, s1 = quant(wa[H:2 * H])
        q2, s2 = quant(f32(inputs[f"W{t}b"]).reshape(H, -1))
        put(f"AmB{t}", q0)
        put(f"Bm{t}", q1)
        put(f"Wb{t}", q2)
        put(f"ws{t}", np.concatenate([s0, s1, s2]))
        put(f"ba{t}", f32(inputs[f"b{t}a"]).reshape(H))
        put(f"bb{t}", f32(inputs[f"b{t}b"]).reshape(-1))
    return blob.reshape(1, BLOB_BYTES)


def make_in_maps(inputs):
    return [{"blob": pack_blob(inputs, c)} for c in range(CORES)]


def run(inputs, trace=False):
    from concourse.bass_utils import run_bass_kernel_spmd
    nc = build()
    in_maps = make_in_maps(inputs)
    res = run_bass_kernel_spmd(nc, in_maps, list(range(CORES)), trace=trace)
    out = np.concatenate(
        [np.asarray(res.results[c]["out"], dtype=np.float32) for c in range(CORES)],
        axis=0)
    return out, res


def kernel(**inputs):
    out, _ = run(inputs, trace=False)
    return out
